# revision 1
# baseline (speedup 1.0000x reference)
"""Trainium2 Bass kernel for nn_MultiHeadAttention_8667244003725.

B=4, S=1024, E=1024, H=16, D=64.  Reference:
  q/k/v = einsum('bse,hed->bhsd', x, W{q,k,v})
  scores = q@k^T/sqrt(D), causal mask, softmax
  heads -> concat (B,S,E);  out = W_O @ concat  (contracts over SEQUENCE dim)
  returns (B, E, E).

Sharding: 8 cores = 4 batches x 2 head-groups (8 heads each).  Because the
output projection contracts over the sequence dim, sharding heads shards the
output columns: core c computes out[b, :, 512*g : 512*g+512] with b=c//2,
g=c%2.  No collectives.

Per-core pipeline (all matmuls in float32r, 1 cyc/row at N>=256):
  xT    = x[b]^T via PE transposes                  [e, s]
  QT/KT = (Wq|Wk pair)^T @ xT   packed 2 heads/matmul -> [2*64, s]
  V_all = xT^T @ Wv_all  -> natural [s, 8*64], stored [s, 8*(64+1)] with a
          ones column per head (row-sum trick)
  per head, per 512-wide q chunk:
     ST[k,q] = KT^T-block @ QT  (PSUM), +causal mask on diagonal blocks,
     P = exp(ST/8)  (no max subtraction; scores/8 <= ~6, fp32-safe),
     OT[65,q] += [V_k|1]^T @ P  accumulated over k blocks
  OT -> transpose 128-col blocks back to q-major [q, 64|l], divide by l,
  write into C[s, 512];  out-slice = W_O^T-chunks^T @ C chunks.
"""

import sys

if '/opt/trn_rl_repo' not in sys.path:
    sys.path.insert(0, '/opt/trn_rl_repo')

import numpy as np

import concourse.bass as bass
import concourse.mybir as mybir
import concourse.tile as tile
from concourse.masks import make_identity

F32 = mybir.dt.float32
F32R = mybir.dt.float32r
AF = mybir.ActivationFunctionType

S = 1024          # sequence
E = 1024          # embed
D = 64            # head dim
HC = 8            # heads per core
NO = 512          # output columns per core
NEG = -1.0e30


def _split_sync_waits(nc, limit=1):
    """The walrus build in this env rejects >1 sem-wait per instruction.
    Hoist excess waits onto preceding same-engine no-ops (same queue, so
    program order preserves the wait semantics)."""
    n = 0
    for f in nc.m.functions:
        for bb in f.blocks:
            out = []
            for ins in bb.instructions:
                si = ins.sync_info
                waits = list(si.on_wait) if si is not None else []
                if len(waits) > limit:
                    excess, keep = waits[:-limit], waits[-limit:]
                    for i in range(0, len(excess), limit):
                        grp = excess[i:i + limit]
                        n += 1
                        out.append(mybir.InstNoOp(
                            name=f'I-synsplit-{n}', ins=[], outs=[],
                            engine=ins.engine,
                            sync_info=mybir.SyncInfo(on_wait=list(grp),
                                                     on_update=[])))
                    si.on_wait = keep
                out.append(ins)
            bb.instructions = out
    return n


def build_nc(split_waits=True):
    nc = bass.Bass()
    BF = mybir.dt.bfloat16
    xb = nc.dram_tensor('xb', [E, S], BF, kind='ExternalInput')   # x[b]^T
    wq = nc.dram_tensor('wq', [E, HC * D], BF, kind='ExternalInput')
    wk = nc.dram_tensor('wk', [E, HC * D], BF, kind='ExternalInput')
    wv = nc.dram_tensor('wv', [E, HC * D], BF, kind='ExternalInput')
    wo = nc.dram_tensor('wo', [E, E], BF, kind='ExternalInput')   # W_O^T
    out = nc.dram_tensor('out', [E, NO], BF, kind='ExternalOutput')

    with tile.TileContext(nc) as tc:
        _emit(nc, tc, xb, wq, wk, wv, wo, out)
    if split_waits:
        _split_sync_waits(nc)
    return nc


def _emit(nc, tc, xb, wq, wk, wv, wo, out):
    BF = mybir.dt.bfloat16

    with (
        tc.tile_pool(name='const', bufs=1) as constp,
        tc.tile_pool(name='bigT', bufs=2) as bigT,      # xTall + WOTall
        tc.tile_pool(name='qk', bufs=1) as qkp,
        tc.tile_pool(name='vall', bufs=1) as vallp,
        tc.tile_pool(name='cbuf', bufs=1) as cp,
        tc.tile_pool(name='psA', bufs=2, space='PSUM') as psA,   # [128,512] mm
        tc.tile_pool(name='psB', bufs=2, space='PSUM') as psB,   # ot accum
        tc.tile_pool(name='psC', bufs=2, space='PSUM') as psC,   # transposes
    ):
        # ---- PE warm-up: junk matmuls keep the HAM clock gate from
        # idling at 1.2 GHz while the input DMAs trickle in.  They use a
        # psC (transpose-pool) buffer whose first real reuse is ~60us
        # later, plus an explicit guard read before any reuse.
        junkt = constp.tile([128, 128], BF, tag='junkt')
        nc.gpsimd.memset(junkt[:], 0.001)
        scrapj = constp.tile([1, 1], F32, tag='scrapj')
        jt = psC.tile([128, 128], F32, tag='tp', name='junkps')
        for _ in range(28):
            nc.tensor.matmul(jt[:], junkt[:], junkt[:],
                             start=True, stop=True)
        nc.vector.tensor_copy(scrapj[:], jt[0:1, 0:1])

        # ---- constants ----------------------------------------------------
        ident = constp.tile([128, 128], BF, tag='ident')
        make_identity(nc, ident[:])
        identf = constp.tile([128, 128], F32, tag='identf')
        make_identity(nc, identf[:])
        ones8 = constp.tile([128, 8], BF, tag='ones8')
        nc.gpsimd.memset(ones8[:], 1.0)
        # multiplicative causal mask for the [128,128] diagonal corner:
        # tri[k, q] = 1 where q >= k else 0
        tri = constp.tile([128, 128], BF, tag='tri')
        nc.gpsimd.memset(tri[:], 1.0)
        nc.gpsimd.affine_select(
            out=tri[:], in_=tri[:], compare_op=mybir.AluOpType.is_ge,
            fill=0.0, base=0, channel_multiplier=-1, pattern=[[1, 128]])

        # ---- xT (bf16, host-transposed + pre-cast): per-chunk DMAs so
        # consumers trickle-start as each chunk lands
        xTall = bigT.tile([128, 8 * S], BF, tag='bigT', name='xTall')
        for ec in range(8):
            nc.sync.dma_start(xTall[:, ec * S:(ec + 1) * S],
                              xb[ec * 128:(ec + 1) * 128, :])
        xT = [xTall[:, ec * S:(ec + 1) * S] for ec in range(8)]

        wpool = tc.tile_pool(name='wts', bufs=1)
        wp = wpool.__enter__()
        # ---- weights (cast to bf16) --------------------------------------
        wqt, wkt, wvt = [], [], []
        for qi, (lst, src, nm) in enumerate(((wqt, wq, 'wq'), (wkt, wk, 'wk'),
                                             (wvt, wv, 'wv'))):
            wall = wp.tile([128, 8 * HC * D], BF, tag=f'{nm}all',
                           name=f'{nm}all')
            for ec in range(8):
                sl = wall[:, ec * HC * D:(ec + 1) * HC * D]
                nc.scalar.dma_start(sl, src[ec * 128:(ec + 1) * 128, :])
                lst.append(sl)
        # warm the ACT exp table (after the weight triggers: the 1.3us
        # table load must not delay them; first real exp is ~30us in)
        warm = constp.tile([1, 2], F32, tag='warm')
        nc.scalar.activation(warm[:], ones8[0:1, 0:2], AF.Exp, scale=0.125)

        # ---- WOT (bf16): W_O^T triggers ride the scalar queue BEHIND the
        # qkv weight triggers (fires ~17us, needed ~120us) so its 2MB
        # doesn't steal ring bandwidth during the critical input window
        WOTall = bigT.tile([128, 8 * E], BF, tag='bigT', name='WOTall')
        for sc in range(8):
            nc.scalar.dma_start(WOTall[:, sc * E:(sc + 1) * E],
                                wo[sc * 128:(sc + 1) * 128, :])
        WOT = [WOTall[:, sc * E:(sc + 1) * E] for sc in range(8)]

        # ---- QKV ----------------------------------------------------------
        # QT/KT packed head pairs: QT2[p][0:64,:] = head 2p, [64:128,:] = 2p+1
        QT2 = [qkp.tile([128, S], BF, tag=f'q{p}', name=f'QT2_{p}')
               for p in range(4)]
        KT2 = [qkp.tile([128, S], BF, tag=f'k{p}', name=f'KT2_{p}')
               for p in range(4)]
        # Q jobs first (all use wq, which is DMA'd first), then K jobs.
        # ec-major across 4 concurrent psum groups so matmuls trickle in as
        # each weight chunk arrives instead of stalling per-job.
        qkv_jobs = [(QT2[p], wqt, p) for p in range(4)] + \
                   [(KT2[p], wkt, p) for p in range(4)]
        for base in range(0, len(qkv_jobs), 2):
            chunk = qkv_jobs[base:base + 2]
            pss = {}
            for ci, (dst, wt, p) in enumerate(chunk):
                pool_, tag_ = (psA, 'mm') if ci == 0 else (psB, 'ot')
                pss[ci] = [pool_.tile([128, 512], F32, tag=tag_,
                                      name=f'qk_{base + ci}_{sc}')
                           for sc in range(2)]
            for ec in range(8):
                for ci, (dst, wt, p) in enumerate(chunk):
                    for sc in range(2):
                        nc.tensor.matmul(
                            pss[ci][sc][:],
                            wt[ec][:, p * 128:(p + 1) * 128],
                            xT[ec][:, sc * 512:(sc + 1) * 512],
                            start=(ec == 0), stop=(ec == 7))
            for ci, (dst, wt, p) in enumerate(chunk):
                for sc in range(2):
                    nc.vector.tensor_copy(dst[:, sc * 512:(sc + 1) * 512],
                                          pss[ci][sc][:])

        # V natural [s, 8*(64+1)] bf16: per head 64 value cols + a ones col
        Vall = [vallp.tile([128, HC * (D + 1)], BF, tag=f'v{st}',
                           name=f'Vall{st}') for st in range(8)]
        for st in range(8):
            ps = psA.tile([128, 512], F32, tag='mm')
            for ec in range(8):
                nc.tensor.matmul(ps[:],
                                 xT[ec][:, st * 128:(st + 1) * 128],
                                 wvt[ec],
                                 start=(ec == 0), stop=(ec == 7))
            v3 = Vall[st][:].rearrange('p (h d) -> p h d', h=HC)
            nc.vector.tensor_copy(v3[:, :, 0:D],
                                  ps[:].rearrange('p (h d) -> p h d', h=HC))
            nc.vector.tensor_copy(v3[:, :, D:D + 1],
                                  ones8[:].rearrange('p (h o) -> p h o', o=1))
        wpool.__exit__(None, None, None)

        # ---- attention + C ------------------------------------------------
        C = [cp.tile([128, NO], BF, tag=f'c{st}', name=f'C{st}')
             for st in range(8)]
        apool = tc.tile_pool(name='attn', bufs=4)
        sstr = apool.__enter__()
        for p in range(4):
            heads = (2 * p, 2 * p + 1)
            QTh = {h: QT2[p][64 * (h % 2):64 * (h % 2) + 64, :] for h in heads}
            KTh = {h: KT2[p][64 * (h % 2):64 * (h % 2) + 64, :] for h in heads}
            for qc in range(2):
                nkb = 4 * qc + 4
                ots_ = {h: psB.tile([128, 512], F32, tag='ot',
                                    name=f'ot_{h}_{qc}') for h in heads}
                for t in range(0, nkb, 2):
                    kbs = (t, t + 1)
                    # per head: both kb score blocks into one 2-bank psum
                    # tile, one exp over the pair, two OT accumulations
                    for h in heads:
                        stp = psA.tile([128, 1024], F32, tag='mm',
                                       name=f'stp_{h}_{qc}_{t}')
                        pexp = sstr.tile([128, 1024], BF, tag='pexp',
                                         name=f'pexp_{h}_{qc}_{t}')
                        offs = []
                        for sl, kb in enumerate(kbs):
                            j = kb - 4 * qc
                            off = 128 * j if j >= 0 else 0
                            W = 512 - off
                            offs.append((sl, kb, j, off, W))
                            nc.tensor.matmul(
                                stp[:128, sl * 512:sl * 512 + W],
                                KTh[h][:, kb * 128:(kb + 1) * 128],
                                QTh[h][:, qc * 512 + off:(qc + 1) * 512],
                                start=True, stop=True)
                        w0, w1 = offs[0][4], offs[1][4]
                        if w0 == 512:
                            nc.scalar.activation(pexp[:, 0:512 + w1],
                                                 stp[:128, 0:512 + w1],
                                                 AF.Exp, scale=0.125)
                        else:
                            nc.scalar.activation(pexp[:, 0:w0],
                                                 stp[:128, 0:w0],
                                                 AF.Exp, scale=0.125)
                            nc.scalar.activation(pexp[:, 512:512 + w1],
                                                 stp[:128, 512:512 + w1],
                                                 AF.Exp, scale=0.125)
                        for sl, kb, j, off, W in offs:
                            if j >= 0:
                                nc.vector.tensor_mul(
                                    pexp[:, sl * 512:sl * 512 + 128],
                                    pexp[:, sl * 512:sl * 512 + 128], tri[:])
                        for sl, kb, j, off, W in offs:
                            nc.tensor.matmul(
                                ots_[h][:D + 1, off:512],
                                Vall[kb][:, h * (D + 1):(h + 1) * (D + 1)],
                                pexp[:, sl * 512:sl * 512 + W],
                                start=(kb == 0), stop=(kb == nkb - 1))
                # transpose [65,128] blocks back to q-major (f32, keeps l
                # exact): cols 0..63 = O rows, col 64 = l; then C = O / l
                for h in heads:
                    ot = ots_[h]
                    ots = sstr.tile([D + 1, 512], F32, tag='ots')
                    nc.vector.tensor_copy(ots[:], ot[:D + 1, :])
                    for qb in range(4):
                        tp = psC.tile([128, 128], F32, tag='tp')
                        nc.tensor.transpose(tp[:, :D + 1],
                                            ots[:, qb * 128:(qb + 1) * 128],
                                            identf[:D + 1, :D + 1])
                        rl = sstr.tile([128, 1], F32, tag='rl')
                        nc.vector.reciprocal(rl[:], tp[:, D:D + 1])
                        nc.vector.tensor_scalar_mul(
                            C[qc * 4 + qb][:, h * D:(h + 1) * D],
                            tp[:, 0:D], rl[:])
        apool.__exit__(None, None, None)

        with tc.tile_pool(name='ostr', bufs=3) as ostr:
            # ---- output projection ---------------------------------------
            for it in range(8):
                ps = psA.tile([128, 512], F32, tag='mm')
                for sc in range(8):
                    nc.tensor.matmul(ps[:],
                                     WOT[sc][:, it * 128:(it + 1) * 128],
                                     C[sc][:],
                                     start=(sc == 0), stop=(sc == 7))
                ys = ostr.tile([128, NO], BF, tag='ys')
                nc.vector.tensor_copy(ys[:], ps[:])
                nc.sync.dma_start(out[it * 128:(it + 1) * 128, :], ys[:])


_NC_CACHE = None


def _get_nc():
    global _NC_CACHE
    if _NC_CACHE is None:
        _NC_CACHE = build_nc()
    return _NC_CACHE


def make_in_maps(x, Wq, Wk, Wv, W_O):
    import ml_dtypes
    bf = ml_dtypes.bfloat16
    x = np.asarray(x, np.float32)
    xT_by_b = [np.ascontiguousarray(x[b].T.astype(bf)) for b in range(4)]
    W_O = np.ascontiguousarray(np.asarray(W_O, np.float32).T.astype(bf))
    in_maps = []
    for c in range(8):
        b, g = c // 2, c % 2
        hsl = slice(HC * g, HC * g + HC)
        in_maps.append({
            'xb': xT_by_b[b],
            'wq': np.ascontiguousarray(
                np.asarray(Wq, np.float32)[hsl].transpose(1, 0, 2)
                .reshape(E, HC * D).astype(bf)),
            'wk': np.ascontiguousarray(
                np.asarray(Wk, np.float32)[hsl].transpose(1, 0, 2)
                .reshape(E, HC * D).astype(bf)),
            'wv': np.ascontiguousarray(
                np.asarray(Wv, np.float32)[hsl].transpose(1, 0, 2)
                .reshape(E, HC * D).astype(bf)),
            'wo': W_O,
        })
    return in_maps


def kernel(x, Wq, Wk, Wv, W_O):
    from concourse.bass_utils import run_bass_kernel_spmd
    nc = _get_nc()
    in_maps = make_in_maps(x, Wq, Wk, Wv, W_O)
    res = run_bass_kernel_spmd(nc, in_maps, list(range(8)))
    full = np.empty((4, E, E), np.float32)
    for c in range(8):
        b, g = c // 2, c % 2
        full[b, :, NO * g:NO * g + NO] = res.results[c]['out']
    return full



# revision 15
# speedup vs baseline: 1.0677x; 1.0677x over previous
"""Trainium2 Bass kernel for nn_MultiHeadAttention_8667244003725.

B=4, S=1024, E=1024, H=16, D=64.  Reference:
  q/k/v = einsum('bse,hed->bhsd', x, W{q,k,v})
  scores = q@k^T/sqrt(D), causal mask, softmax
  heads -> concat (B,S,E);  out = W_O @ concat  (contracts over SEQUENCE dim)
  returns (B, E, E).

Sharding: 8 cores = 4 batches x 2 head-groups (8 heads each).  Because the
output projection contracts over the sequence dim, sharding heads shards the
output columns: core c computes out[b, :, 512*g : 512*g+512] with b=c//2,
g=c%2.  No collectives.

v2 layout: software-pipelined so the PE never idles (keeps the HAM clock
gate at 2.4 GHz through the attention phase, which ran at 1.2 GHz in v1):
  Q0,K0 dense -> attention(p) runs with independent filler matmuls
  interleaved into its dependency-wait slots:
    attn(0) <- V-projection MMs,  attn(p) <- Q/K(p+1) MMs,
    attn(3,qc=1) <- junk MMs,  attn(3,qc=0) <- outproj first half (st 4..7).
  Output projection is split in halves (st4..7 accumulated early into an
  SBUF carry ACCP via ACT copies; st0..3 + combine at the tail).
Engine rebalance: causal tri-mask mul on GpSimd (was DVE), per-head-chunk
reciprocals batched [128,4], C normalization as one broadcast
scalar_tensor_tensor per (head, qc) (was 8 DVE ops).
"""

import sys

if '/opt/trn_rl_repo' not in sys.path:
    sys.path.insert(0, '/opt/trn_rl_repo')

from collections import deque

import numpy as np

import concourse.bass as bass
import concourse.mybir as mybir
import concourse.tile as tile
from concourse.masks import make_identity

F32 = mybir.dt.float32
BF = mybir.dt.bfloat16
AF = mybir.ActivationFunctionType
ALU = mybir.AluOpType

S = 1024          # sequence
E = 1024          # embed
D = 64            # head dim
HC = 8            # heads per core
NO = 512          # output columns per core


def _split_sync_waits(nc, limit=1):
    """The walrus build in this env rejects >1 sem-wait per instruction.
    Hoist excess waits onto preceding same-engine no-ops (same queue, so
    program order preserves the wait semantics)."""
    n = 0
    for f in nc.m.functions:
        for bb in f.blocks:
            out = []
            for ins in bb.instructions:
                si = ins.sync_info
                waits = list(si.on_wait) if si is not None else []
                if len(waits) > limit:
                    excess, keep = waits[:-limit], waits[-limit:]
                    for i in range(0, len(excess), limit):
                        grp = excess[i:i + limit]
                        n += 1
                        out.append(mybir.InstNoOp(
                            name=f'I-synsplit-{n}', ins=[], outs=[],
                            engine=ins.engine,
                            sync_info=mybir.SyncInfo(on_wait=list(grp),
                                                     on_update=[])))
                    si.on_wait = keep
                out.append(ins)
            bb.instructions = out
    return n


def build_nc(split_waits=True):
    nc = bass.Bass()
    xb = nc.dram_tensor('xb', [E, S], BF, kind='ExternalInput')   # x[b]^T
    wq = nc.dram_tensor('wq', [E, HC * D], BF, kind='ExternalInput')
    wk = nc.dram_tensor('wk', [E, HC * D], BF, kind='ExternalInput')
    wv = nc.dram_tensor('wv', [E, HC * D], BF, kind='ExternalInput')
    wo = nc.dram_tensor('wo', [E, E], BF, kind='ExternalInput')   # W_O^T
    out = nc.dram_tensor('out', [E, NO], BF, kind='ExternalOutput')

    with tile.TileContext(nc) as tc:
        _emit(nc, tc, xb, wq, wk, wv, wo, out)
    if split_waits:
        _split_sync_waits(nc)
    return nc


def _emit(nc, tc, xb, wq, wk, wv, wo, out):
    with (
        tc.tile_pool(name='const', bufs=1) as constp,
        tc.tile_pool(name='bigT', bufs=2) as bigT,      # xTall + WOTall
        tc.tile_pool(name='wts', bufs=1) as wp,
        tc.tile_pool(name='qk', bufs=1) as qkp,
        tc.tile_pool(name='vall', bufs=1) as vallp,
        tc.tile_pool(name='cbuf', bufs=1) as cp,
        tc.tile_pool(name='accp', bufs=1) as accpp,
        tc.tile_pool(name='attn', bufs=4) as sstr,
        tc.tile_pool(name='ostr', bufs=3) as ostr,
        tc.tile_pool(name='psQK', bufs=2, space='PSUM') as psQK,  # 512 mm
        tc.tile_pool(name='psS', bufs=2, space='PSUM') as psS,    # scores
        tc.tile_pool(name='psOT', bufs=2, space='PSUM') as psOT,  # ot accum
        tc.tile_pool(name='psT', bufs=2, space='PSUM') as psT,    # transposes
    ):
        # ---- PE warm-up: junk matmuls keep the HAM clock gate from
        # idling at 1.2 GHz while the input DMAs trickle in.
        junkt = constp.tile([128, 128], BF, tag='junkt')
        nc.gpsimd.memset(junkt[:], 0.001)
        scrapj = constp.tile([1, 1], F32, tag='scrapj')
        jt = psT.tile([128, 260], F32, tag='tp', name='junkps')
        for _ in range(14):
            nc.tensor.matmul(jt[:, 0:128], junkt[:], junkt[:],
                             start=True, stop=True)

        # ---- constants (gpsimd; must precede the gpsimd DMA triggers) ----
        identf = constp.tile([128, 128], F32, tag='identf')
        make_identity(nc, identf[:])
        ones8 = constp.tile([128, 8], BF, tag='ones8')
        nc.gpsimd.memset(ones8[:], 1.0)
        # multiplicative causal mask for the [128,128] diagonal corner:
        # tri[k, q] = 1 where q >= k else 0
        tri = constp.tile([128, 128], BF, tag='tri')
        nc.gpsimd.memset(tri[:], 1.0)
        nc.gpsimd.affine_select(
            out=tri[:], in_=tri[:], compare_op=ALU.is_ge,
            fill=0.0, base=0, channel_multiplier=-1, pattern=[[1, 128]])

        # ---- input DMA. Trigger instructions cost ~600ns each on the
        # issuing engine and DMAs can only start from SP/ACT/gpsimd, so
        # spread them over three queues and keep the ACT queue nearly
        # clear for the attention exps (in v1 all weight triggers rode
        # the scalar queue and the first exp could not issue until ~31us).
        # sync: xT+wk interleaved (paced for the ec-major Q0/K0
        # consumption) then wo behind; gpsimd: wq; scalar: wv then warm.
        xTall = bigT.tile([128, 8 * S], BF, tag='bigT', name='xTall')
        wqall = wp.tile([128, 8 * HC * D], BF, tag='wqall', name='wqall')
        wkall = wp.tile([128, 8 * HC * D], BF, tag='wkall', name='wkall')
        wvall = wp.tile([128, 8 * HC * D], BF, tag='wvall', name='wvall')
        for ec in range(8):
            nc.sync.dma_start(xTall[:, ec * S:(ec + 1) * S],
                              xb[ec * 128:(ec + 1) * 128, :])
            nc.sync.dma_start(wkall[:, ec * HC * D:(ec + 1) * HC * D],
                              wk[ec * 128:(ec + 1) * 128, :])
            nc.gpsimd.dma_start(wqall[:, ec * HC * D:(ec + 1) * HC * D],
                                wq[ec * 128:(ec + 1) * 128, :])
            nc.scalar.dma_start(wvall[:, ec * HC * D:(ec + 1) * HC * D],
                                wv[ec * 128:(ec + 1) * 128, :])
        xT = [xTall[:, ec * S:(ec + 1) * S] for ec in range(8)]
        wqt = [wqall[:, ec * HC * D:(ec + 1) * HC * D] for ec in range(8)]
        wkt = [wkall[:, ec * HC * D:(ec + 1) * HC * D] for ec in range(8)]
        wvt = [wvall[:, ec * HC * D:(ec + 1) * HC * D] for ec in range(8)]
        nc.vector.tensor_copy(scrapj[:], jt[0:1, 0:1])  # close junk writes

        # warm the ACT exp table; W_O^T triggers ride the sync queue
        # BEHIND xT+wk (W_O data is not needed until ~85us, and this
        # keeps its 2MB from stealing HBM bandwidth in the lead-in)
        warm = constp.tile([1, 2], F32, tag='warm')
        nc.scalar.activation(warm[:], ones8[0:1, 0:2], AF.Exp, scale=0.125)
        WOTall = bigT.tile([128, 8 * E], BF, tag='bigT', name='WOTall')
        for sc in range(8):
            nc.sync.dma_start(WOTall[:, sc * E:(sc + 1) * E],
                              wo[sc * 128:(sc + 1) * 128, :])
        WOT = [WOTall[:, sc * E:(sc + 1) * E] for sc in range(8)]

        # ---- SBUF destinations -------------------------------------------
        QT2 = [qkp.tile([128, S], BF, tag=f'q{p}', name=f'QT2_{p}')
               for p in range(4)]
        KT2 = [qkp.tile([128, S], BF, tag=f'k{p}', name=f'KT2_{p}')
               for p in range(4)]
        Vall = [vallp.tile([128, HC * (D + 1)], BF, tag=f'v{st}',
                           name=f'Vall{st}') for st in range(8)]
        Cbig = cp.tile([128, 8 * NO], BF, tag='cbig', name='Cbig')
        cb3 = Cbig[:].rearrange('p (st c) -> p st c', c=NO)
        ACCP = accpp.tile([128, 8 * NO], F32, tag='accp', name='ACCP')

        # ---- filler infrastructure ---------------------------------------
        # Units are (gid, fn) closures emitting a couple of independent PE
        # matmuls; attention emission drains them into its dependency-wait
        # slots so the PE queue never runs dry (keeps HAM at K=8/8).
        units = deque()
        junk_on = [False]
        junk_pool = [None]

        def emit_junk():
            # fresh tile each call: the pool's WAR/WAW tracking keeps the
            # junk writes ordered against that buffer's previous users.
            # attn(3, qc=1) borrows the idle psQK rotation; the tail uses
            # psS (psQK may still have in-flight ACT readers there).
            pool, tag = junk_pool[0]
            jq = pool.tile([128, 512], F32, tag=tag, name='junkfill')
            nc.tensor.matmul(jq[:, 0:128], junkt[:], junkt[:],
                             start=True, stop=True)
            nc.tensor.matmul(jq[:, 128:256], junkt[:], junkt[:],
                             start=True, stop=True)

        def fill(n):
            for _ in range(n):
                if units:
                    units.popleft()[1]()
                elif junk_on[0]:
                    emit_junk()

        def drain_upto(gid):
            while units and units[0][0] <= gid:
                units.popleft()[1]()

        def drain_all():
            while units:
                units.popleft()[1]()

        # ---- QK jobs ------------------------------------------------------
        def qk_units(p):
            """Units computing QT2[p] / KT2[p] (each: 2 accumulation MMs or
            the psum->sbuf cast)."""
            us = []
            for dst, wt in ((QT2[p], wqt), (KT2[p], wkt)):
                for sc in range(2):
                    box = {}

                    def mk_mm(ec0, dst=dst, wt=wt, sc=sc, box=box):
                        def go():
                            if 'ps' not in box:
                                box['ps'] = psQK.tile([128, 512], F32,
                                                      tag='qk', name='qkps')
                            for ec in (ec0, ec0 + 1):
                                nc.tensor.matmul(
                                    box['ps'][:],
                                    wt[ec][:, p * 128:(p + 1) * 128],
                                    xT[ec][:, sc * 512:(sc + 1) * 512],
                                    start=(ec == 0), stop=(ec == 7))
                        return go

                    def mk_cp(dst=dst, sc=sc, box=box):
                        def go():
                            nc.vector.tensor_copy(
                                dst[:, sc * 512:(sc + 1) * 512],
                                box['ps'][:])
                        return go

                    for ec0 in range(0, 8, 2):
                        us.append((100 + p, mk_mm(ec0)))
                    us.append((100 + p, mk_cp()))
            return us

        def v_units():
            """Units computing Vall[st] (value proj + ones column)."""
            us = []
            for st in range(8):
                box = {}

                def mk_mm(ec0, st=st, box=box):
                    def go():
                        if 'ps' not in box:
                            box['ps'] = psQK.tile([128, 512], F32, tag='qk',
                                                  name='vps')
                        for ec in (ec0, ec0 + 1):
                            nc.tensor.matmul(
                                box['ps'][:],
                                xT[ec][:, st * 128:(st + 1) * 128],
                                wvt[ec],
                                start=(ec == 0), stop=(ec == 7))
                    return go

                def mk_cp(st=st, box=box):
                    def go():
                        v3 = Vall[st][:].rearrange('p (h d) -> p h d', h=HC)
                        nc.vector.tensor_copy(
                            v3[:, :, 0:D],
                            box['ps'][:].rearrange('p (h d) -> p h d', h=HC))
                        nc.vector.tensor_copy(
                            v3[:, :, D:D + 1],
                            ones8[:].rearrange('p (h o) -> p h o', o=1))
                    return go

                for ec0 in range(0, 8, 2):
                    us.append((st, mk_mm(ec0)))
                us.append((st, mk_cp()))
            return us

        def outproj_half_units(sts, first_half):
            """Units accumulating out[it] over C chunks sts; first half
            parks the psum into ACCP (ACT copy), second half combines with
            ACCP and DMAs out."""
            us = []
            for it in range(8):
                box = {}

                def mk_mm(k, it=it, box=box):
                    def go():
                        if 'ps' not in box:
                            box['ps'] = psQK.tile([128, 512], F32, tag='qk',
                                                  name=f'op{sts[0]}_{it}')
                        for st in (sts[k], sts[k + 1]):
                            nc.tensor.matmul(
                                box['ps'][:],
                                WOT[st][:, it * 128:(it + 1) * 128],
                                Cbig[:, st * NO:(st + 1) * NO],
                                start=(st == sts[0]), stop=(st == sts[-1]))
                    return go

                def mk_fin(it=it, box=box):
                    def go():
                        if first_half:
                            nc.scalar.copy(
                                ACCP[:, it * NO:(it + 1) * NO], box['ps'][:])
                        else:
                            ys = ostr.tile([128, NO], BF, tag='ys')
                            nc.vector.tensor_add(
                                ys[:], box['ps'][:],
                                ACCP[:, it * NO:(it + 1) * NO])
                            nc.sync.dma_start(
                                out[it * 128:(it + 1) * 128, :], ys[:])
                    return go

                us.append((200 + it, mk_mm(0)))
                us.append((200 + it, mk_mm(2)))
                us.append((200 + it, mk_fin()))
            return us

        # ---- attention ----------------------------------------------------
        def attention(p, qc):
            heads = (2 * p, 2 * p + 1)
            QTh = {h: QT2[p][64 * (h % 2):64 * (h % 2) + 64, :]
                   for h in heads}
            KTh = {h: KT2[p][64 * (h % 2):64 * (h % 2) + 64, :]
                   for h in heads}
            nkb = 8 if qc == 1 else 4
            ots = {h: psOT.tile([128, 512], F32, tag='ot',
                                name=f'ot_{h}_{qc}') for h in heads}
            for kb in range(nkb):
                j = kb - 4 * qc
                off = 128 * j if j >= 0 else 0
                W = 512 - off
                is_diag = j >= 0
                if p == 0:
                    drain_upto(kb)   # V(st<=kb) must precede OT(kb)
                sps, pex = {}, {}
                for h in heads:
                    sps[h] = psS.tile([128, 512], F32, tag='s',
                                      name=f's_{h}_{qc}_{kb}')
                    nc.tensor.matmul(
                        sps[h][:, 0:W],
                        KTh[h][:, kb * 128:(kb + 1) * 128],
                        QTh[h][:, qc * 512 + off:(qc + 1) * 512],
                        start=True, stop=True)
                fill(1)
                for h in heads:
                    pex[h] = sstr.tile([128, 512], BF, tag='pexp',
                                       name=f'pex_{h}_{qc}_{kb}')
                    nc.scalar.activation(pex[h][:, 0:W], sps[h][:, 0:W],
                                         AF.Exp, scale=0.125)
                    if is_diag:
                        nc.gpsimd.tensor_mul(pex[h][:, 0:128],
                                             pex[h][:, 0:128], tri[:])
                fill(1)
                for h in heads:
                    nc.tensor.matmul(
                        ots[h][:D + 1, off:512],
                        Vall[kb][:, h * (D + 1):(h + 1) * (D + 1)],
                        pex[h][:, 0:W],
                        start=(kb == 0), stop=(kb == nkb - 1))
                fill(2)
            # transpose [65,512] back to q-major, normalize by l, write C
            for h in heads:
                osb = sstr.tile([D + 1, 512], F32, tag='ots')
                nc.vector.tensor_copy(osb[:], ots[h][:D + 1, :])
                tpT = psT.tile([128, 4 * (D + 1)], F32, tag='tp')
                for qb in range(4):
                    nc.tensor.transpose(
                        tpT[:, qb * 65:qb * 65 + 65],
                        osb[:, qb * 128:(qb + 1) * 128],
                        identf[:D + 1, :D + 1])
                fill(1)
                tp3 = tpT[:].rearrange('p (a c) -> p a c', c=D + 1)
                rl = sstr.tile([128, 4], F32, tag='rl')
                rl3 = rl[:].rearrange('p (a c) -> p a c', c=1)
                nc.vector.reciprocal(rl3[:, :, :], tp3[:, :, D:D + 1])
                nc.vector.scalar_tensor_tensor(
                    out=cb3[:, 4 * qc:4 * qc + 4, h * D:(h + 1) * D],
                    in0=tp3[:, :, 0:D],
                    scalar=1.0,
                    in1=rl3.broadcast_to([128, 4, D]),
                    op0=ALU.mult, op1=ALU.mult)
                fill(1)

        # ---- emission schedule -------------------------------------------
        # Q0, K0 dense (PE warms up / DMA still arriving)
        for _, fn in qk_units(0):
            fn()
        # V and QK(1) both ride the DMA-bound lead-in window / attn(0)
        units.extend(v_units())
        units.extend(qk_units(1))
        for p in range(4):
            if p < 3:
                # QK(p+1) fills attn(p)'s wait slots (gids 100+: never
                # pulled in by drain_upto, which only targets V units)
                if p > 0:
                    units.extend(qk_units(p + 1))
                attention(p, 1)   # qc=1 first: C[4..7] complete sooner
                attention(p, 0)
                drain_all()
            else:
                # no independent real work left inside attn(3, qc=1):
                junk_pool[0] = (psQK, 'qk')
                junk_on[0] = True
                attention(3, 1)
                junk_on[0] = False
                # first half of the output projection (st 4..7 -> ACCP)
                # fills attn(3, qc=0)'s wait slots
                units.extend(outproj_half_units([4, 5, 6, 7],
                                                first_half=True))
                attention(3, 0)
                drain_all()
        # tail: second half (st 0..3) + combine + DMA out.  A few junk
        # matmuls cover the DVE latency of the last C writes so the PE
        # does not idle (and HAM-throttle) right before the final burst.
        junk_pool[0] = (psS, 's')
        for _ in range(5):
            emit_junk()
        for _, fn in outproj_half_units([0, 1, 2, 3], first_half=False):
            fn()


_NC_CACHE = None


def _get_nc():
    global _NC_CACHE
    if _NC_CACHE is None:
        _NC_CACHE = build_nc()
    return _NC_CACHE


def make_in_maps(x, Wq, Wk, Wv, W_O):
    import ml_dtypes
    bf = ml_dtypes.bfloat16
    x = np.asarray(x, np.float32)
    xT_by_b = [np.ascontiguousarray(x[b].T.astype(bf)) for b in range(4)]
    W_O = np.ascontiguousarray(np.asarray(W_O, np.float32).T.astype(bf))
    in_maps = []
    for c in range(8):
        b, g = c // 2, c % 2
        hsl = slice(HC * g, HC * g + HC)
        in_maps.append({
            'xb': xT_by_b[b],
            'wq': np.ascontiguousarray(
                np.asarray(Wq, np.float32)[hsl].transpose(1, 0, 2)
                .reshape(E, HC * D).astype(bf)),
            'wk': np.ascontiguousarray(
                np.asarray(Wk, np.float32)[hsl].transpose(1, 0, 2)
                .reshape(E, HC * D).astype(bf)),
            'wv': np.ascontiguousarray(
                np.asarray(Wv, np.float32)[hsl].transpose(1, 0, 2)
                .reshape(E, HC * D).astype(bf)),
            'wo': W_O,
        })
    return in_maps


def kernel(x, Wq, Wk, Wv, W_O):
    from concourse.bass_utils import run_bass_kernel_spmd
    nc = _get_nc()
    in_maps = make_in_maps(x, Wq, Wk, Wv, W_O)
    res = run_bass_kernel_spmd(nc, in_maps, list(range(8)))
    full = np.empty((4, E, E), np.float32)
    for c in range(8):
        b, g = c // 2, c % 2
        full[b, :, NO * g:NO * g + NO] = res.results[c]['out']
    return full
